# revision 11
# baseline (speedup 1.0000x reference)
"""Causal self-attention (B=4, T=2048, C=1024, 16 heads, rope) on 8 trn2
NeuronCores, tensor-parallel over heads (2 heads/core).

Each core gets the full token stream plus its head-group's W_attn columns /
W_proj rows, computes a full-shape partial of the output projection, and the
host sums the 8 partials (the all-reduce) and transposes back.

All matmuls run as float32r (full PE rate, ~1e-4 rel err). Scores are
computed transposed ([k, q] layout) so softmax(P) @ V needs no transposes;
the softmax denominator comes from an extra ones-stationary matmul whose
M=64 output is already broadcast across partitions.
"""

import ml_dtypes
import numpy as np

import concourse.bacc as bacc
import concourse.mybir as mybir
import concourse.tile as tile
from concourse.bass_utils import run_bass_kernel_spmd

F32 = mybir.dt.float32
F32R = mybir.dt.float32r
BF16 = mybir.dt.bfloat16
AF = mybir.ActivationFunctionType

B, T, C = 4, 2048, 1024
N_HEAD, HEAD_DIM = 16, 64
N_CORES = 8
HPC = N_HEAD // N_CORES          # heads per core = 2
HF = HPC * HEAD_DIM              # per-core head features = 128
NT = B * T                       # 8192 tokens
KT = C // 128                    # 8 contraction tiles for qkv proj
QC = 512                         # query-chunk width
NQC = T // QC                    # 4 query chunks per batch
ROPE_BASE = 10000.0
SCALE = 1.0 / 8.0                # 1/sqrt(HEAD_DIM)

_PROGRAM = None


def _build_program():
    nc = bacc.Bacc(None, target_bir_lowering=False)

    xT = nc.dram_tensor("xT", [C, NT], F32R, kind="ExternalInput")
    wq = nc.dram_tensor("wq", [C, HF], F32R, kind="ExternalInput")
    wk = nc.dram_tensor("wk", [C, HF], F32R, kind="ExternalInput")
    wv = nc.dram_tensor("wv", [C, HF], F32R, kind="ExternalInput")
    wp = nc.dram_tensor("wp", [HF, C], F32R, kind="ExternalInput")
    identd = nc.dram_tensor("identd", [128, 128], F32R, kind="ExternalInput")
    onesdd = nc.dram_tensor("onesdd", [128, 64], F32R, kind="ExternalInput")
    cosd = nc.dram_tensor("cosd", [HF, T], F32R, kind="ExternalInput")
    ssind = nc.dram_tensor("ssind", [HF, T], F32R, kind="ExternalInput")
    outT = nc.dram_tensor("outT", [C, NT], F32, kind="ExternalOutput")

    with tile.TileContext(nc) as tc:
        with (
            tc.tile_pool(name="const", bufs=1) as cpool,
            tc.tile_pool(name="sx", bufs=2) as sx,
            tc.tile_pool(name="srope", bufs=1) as srope,
            tc.tile_pool(name="sv", bufs=2) as sv,
            tc.tile_pool(name="spt", bufs=4) as spt,
            tc.tile_pool(name="sy", bufs=2) as sy,
            tc.tile_pool(name="sst", bufs=2) as sst,
            tc.tile_pool(name="pmm", bufs=2, space="PSUM") as pmm,
            tc.tile_pool(name="psc", bufs=2, space="PSUM") as psc,
            tc.tile_pool(name="py", bufs=2, space="PSUM") as py,
        ):
            # ---- constants ----
            ident = cpool.tile([128, 128], F32R, tag="ident")
            nc.sync.dma_start(ident[:], identd[:])
            onesd = cpool.tile([128, 64], F32R, tag="onesd")
            nc.sync.dma_start(onesd[:], onesdd[:])

            wqs = cpool.tile([128, C], F32R, tag="wqs")
            wks = cpool.tile([128, C], F32R, tag="wks")
            wvs = cpool.tile([128, C], F32R, tag="wvs")
            for kt in range(KT):
                nc.sync.dma_start(wqs[:, kt * HF:(kt + 1) * HF], wq[kt * 128:(kt + 1) * 128, :])
                nc.sync.dma_start(wks[:, kt * HF:(kt + 1) * HF], wk[kt * 128:(kt + 1) * 128, :])
                nc.sync.dma_start(wvs[:, kt * HF:(kt + 1) * HF], wv[kt * 128:(kt + 1) * 128, :])
            wps = cpool.tile([128, C], F32R, tag="wps")
            nc.sync.dma_start(wps[:], wp[:])
            cost = cpool.tile([128, T], F32R, tag="cost")
            nc.sync.dma_start(cost[:], cosd[:])
            ssint = cpool.tile([128, T], F32R, tag="ssint")
            nc.sync.dma_start(ssint[:], ssind[:])

            for b in range(B):
                boff = b * T
                # ---- qkv projection ([feat, tok] layout) ----
                raw_q = srope.tile([128, T], F32R, tag="raw_q")
                raw_k = srope.tile([128, T], F32R, tag="raw_k")
                vT = srope.tile([128, T], F32R, tag="vT")
                for c in range(NQC):
                    xs = sx.tile([128, KT * QC], F32R, tag="xs")
                    for kt in range(KT):
                        nc.sync.dma_start(
                            xs[:, kt * QC:(kt + 1) * QC],
                            xT[kt * 128:(kt + 1) * 128, boff + c * QC: boff + (c + 1) * QC],
                        )
                    for wslab, dest in ((wqs, raw_q), (wks, raw_k), (wvs, vT)):
                        ps = pmm.tile([128, QC], F32, tag="mm")
                        for kt in range(KT):
                            nc.tensor.matmul(
                                ps[:], wslab[:, kt * HF:(kt + 1) * HF],
                                xs[:, kt * QC:(kt + 1) * QC],
                                start=(kt == 0), stop=(kt == KT - 1),
                            )
                        nc.vector.tensor_copy(dest[:, c * QC:(c + 1) * QC], ps[:])

                # ---- rope on q, k ----
                rope_q = srope.tile([128, T], F32R, tag="rope_q")
                rope_k = srope.tile([128, T], F32R, tag="rope_k")
                for raw, dst in ((raw_q, rope_q), (raw_k, rope_k)):
                    shuf = srope.tile([128, T], F32R, tag="shuf")
                    for h in range(HPC):
                        hb = h * 64
                        nc.sync.dma_start(shuf[hb:hb + 32, :], raw[hb + 32:hb + 64, :])
                        nc.sync.dma_start(shuf[hb + 32:hb + 64, :], raw[hb:hb + 32, :])
                    ta = srope.tile([128, T], F32R, tag="ta")
                    tb = srope.tile([128, T], F32R, tag="tb")
                    nc.vector.tensor_mul(ta[:], raw[:], cost[:])
                    nc.vector.tensor_mul(tb[:], shuf[:], ssint[:])
                    nc.gpsimd.tensor_add(dst[:], ta[:], tb[:])

                # ---- v -> natural layout [tok, feat] + ones cols ----
                # per ki (stride 130): [h0 v (64) | ones | h1 v (64) | ones]
                vv = sv.tile([128, (T // 128) * 130], F32R, tag="vv")
                nc.sync.dma_start(vv[:, 64:(T // 128) * 130:65], onesd[:, 0:32])
                for ki in range(T // 128):
                    pst = pmm.tile([128, QC], F32R, tag="mm")
                    nc.tensor.transpose(pst[:, 0:128], vT[:, ki * 128:(ki + 1) * 128], ident[:])
                    nc.vector.tensor_copy(vv[:, ki * 130:ki * 130 + 64], pst[:, 0:64])
                    nc.vector.tensor_copy(vv[:, ki * 130 + 65:ki * 130 + 129], pst[:, 64:128])

                # ---- attention ----
                ynorm = sy.tile([128, T], F32R, tag="ynorm")
                for qc in range(NQC):
                    qs = qc * QC
                    nki = (qs + QC) // 128
                    y_aug = [py.tile([65, QC], F32, tag="y", name=f"yaug{h}") for h in range(HPC)]
                    for pr in range(nki // 2):
                        kia, kib = 2 * pr, 2 * pr + 1
                        c0a = max(0, kia * 128 - qs)
                        c0b = max(0, kib * 128 - qs)
                        for h in range(HPC):
                            hb = h * 64
                            sc = psc.tile([128, 2 * QC], F32, tag="sc")
                            nc.tensor.matmul(
                                sc[:, c0a:QC],
                                rope_k[hb:hb + 64, kia * 128:(kia + 1) * 128],
                                rope_q[hb:hb + 64, qs + c0a:qs + QC],
                                start=True, stop=True,
                            )
                            nc.tensor.matmul(
                                sc[:, QC + c0b:2 * QC],
                                rope_k[hb:hb + 64, kib * 128:(kib + 1) * 128],
                                rope_q[hb:hb + 64, qs + c0b:qs + QC],
                                start=True, stop=True,
                            )
                            pt = spt.tile([128, 2 * QC], F32R, tag="pt")
                            nc.scalar.activation(
                                pt[:, c0a:2 * QC], sc[:, c0a:2 * QC], AF.Exp, scale=SCALE,
                            )
                            for ki, half, c0 in ((kia, 0, c0a), (kib, 1, c0b)):
                                if ki * 128 >= qs:  # diagonal block: zero where k > q
                                    nc.gpsimd.affine_select(
                                        pt[:, half * QC + c0: half * QC + c0 + 128],
                                        pt[:, half * QC + c0: half * QC + c0 + 128],
                                        pattern=[[1, 128]],
                                        compare_op=mybir.AluOpType.is_ge,
                                        fill=0.0, base=0, channel_multiplier=-1,
                                    )
                                nc.tensor.matmul(
                                    y_aug[h][0:65, c0:QC],
                                    vv[:, ki * 130 + 65 * h: ki * 130 + 65 * h + 65],
                                    pt[:, half * QC + c0: (half + 1) * QC],
                                    start=(pr == 0 and half == 0),
                                    stop=(pr == nki // 2 - 1 and half == 1),
                                )
                    for h in range(HPC):
                        rec = sst.tile([128, QC], F32R, tag="rec")
                        with nc.allow_low_precision(reason="f32r is f32-width"):
                            nc.vector.reciprocal(rec[64:65, :], y_aug[h][64:65, :])
                        bc = pmm.tile([128, QC], F32, tag="mm")
                        nc.tensor.matmul(
                            bc[0:64, :], onesd[64:65, 0:64], rec[64:65, :],
                            start=True, stop=True,
                        )
                        bcs = sst.tile([64, QC], F32, tag="bcs")
                        nc.vector.tensor_copy(bcs[:], bc[0:64, :])
                        if h == 0:
                            nc.vector.tensor_mul(
                                ynorm[0:64, qs:qs + QC], y_aug[h][0:64, :], bcs[:])
                        else:
                            hn = sst.tile([64, QC], F32R, tag="hn")
                            nc.vector.tensor_mul(hn[:], y_aug[h][0:64, :], bcs[:])
                            nc.sync.dma_start(ynorm[64:128, qs:qs + QC], hn[:])

                # ---- output projection (out^T partial) ----
                for of in range(KT):
                    for c in range(NQC):
                        op = pmm.tile([128, QC], F32, tag="mm")
                        nc.tensor.matmul(
                            op[:], wps[:, of * 128:(of + 1) * 128],
                            ynorm[:, c * QC:(c + 1) * QC],
                            start=True, stop=True,
                        )
                        st = sst.tile([128, QC], F32, tag="st")
                        if (of + c) % 2 == 0:
                            nc.vector.tensor_copy(st[:], op[:])
                        else:
                            nc.scalar.copy(st[:], op[:])
                        nc.sync.dma_start(
                            outT[of * 128:(of + 1) * 128, boff + c * QC: boff + (c + 1) * QC],
                            st[:],
                        )
    nc.finalize()
    return nc


def _program():
    global _PROGRAM
    if _PROGRAM is None:
        _PROGRAM = _build_program()
    return _PROGRAM


def _rope_tables():
    inv_freq = 1.0 / (ROPE_BASE ** (np.arange(0, HEAD_DIM, 2, dtype=np.float32) / HEAD_DIM))
    t = np.arange(T, dtype=np.float32)
    freqs = np.outer(t, inv_freq).astype(np.float32)        # [T, 32]
    emb = np.concatenate([freqs, freqs], axis=1)            # [T, 64]
    cos = np.cos(emb).astype(np.float32)                    # [T, 64]
    sin = np.sin(emb).astype(np.float32)
    ssin = sin.copy()
    ssin[:, :32] *= -1.0                                    # signed for rotate_half
    cosT = np.ascontiguousarray(cos.T)                      # [64, T]
    ssinT = np.ascontiguousarray(ssin.T)
    cos2 = np.concatenate([cosT] * HPC, axis=0)             # [128, T]
    ssin2 = np.concatenate([ssinT] * HPC, axis=0)
    return cos2, ssin2


def _prep_in_maps(x, W_attn, W_proj):
    x = np.asarray(x, dtype=np.float32)
    W_attn = np.asarray(W_attn, dtype=np.float32)
    W_proj = np.asarray(W_proj, dtype=np.float32)
    xT = np.ascontiguousarray(x.reshape(NT, C).T)
    cos2, ssin2 = _rope_tables()
    in_maps = []
    for i in range(N_CORES):
        cs = i * HF
        in_maps.append({
            "xT": xT,
            "wq": np.ascontiguousarray(W_attn[:, cs:cs + HF]),
            "wk": np.ascontiguousarray(W_attn[:, C + cs:C + cs + HF]),
            "wv": np.ascontiguousarray(W_attn[:, 2 * C + cs:2 * C + cs + HF]),
            "wp": np.ascontiguousarray(W_proj[cs:cs + HF, :]),
            "identd": np.eye(128, dtype=np.float32),
            "onesdd": np.ones((128, 64), dtype=np.float32),
            "cosd": cos2,
            "ssind": ssin2,
        })
    return in_maps


def _run(in_maps, trace=False, **kwargs):
    return run_bass_kernel_spmd(
        _program(), in_maps, core_ids=list(range(N_CORES)), trace=trace, **kwargs
    )


def kernel(x, W_attn, W_proj):
    in_maps = _prep_in_maps(x, W_attn, W_proj)
    res = _run(in_maps)
    acc = np.zeros((C, NT), dtype=np.float32)
    for r in res.results:
        acc += r["outT"]
    return np.ascontiguousarray(acc.T).reshape(B, T, C)


# revision 12
# speedup vs baseline: 1.0984x; 1.0984x over previous
"""Causal self-attention (B=4, T=2048, C=1024, 16 heads, rope) on 8 trn2
NeuronCores, tensor-parallel over heads (2 heads/core).

Each core gets the full token stream plus its head-group's W_attn columns /
W_proj rows, computes a full-shape partial of the output projection, and the
host sums the 8 partials (the all-reduce) and transposes back.

All matmuls run as float32r (full PE rate, ~1e-4 rel err). Scores are
computed transposed ([k, q] layout) so softmax(P) @ V needs no transposes;
the softmax denominator comes from an extra ones-stationary matmul whose
M=64 output is already broadcast across partitions.
"""

import ml_dtypes
import numpy as np

import concourse.bacc as bacc
import concourse.mybir as mybir
import concourse.tile as tile
from concourse.bass_utils import run_bass_kernel_spmd

F32 = mybir.dt.float32
F32R = mybir.dt.float32r
BF16 = mybir.dt.bfloat16
AF = mybir.ActivationFunctionType

B, T, C = 4, 2048, 1024
N_HEAD, HEAD_DIM = 16, 64
N_CORES = 8
HPC = N_HEAD // N_CORES          # heads per core = 2
HF = HPC * HEAD_DIM              # per-core head features = 128
NT = B * T                       # 8192 tokens
KT = C // 128                    # 8 contraction tiles for qkv proj
QC = 512                         # query-chunk width
NQC = T // QC                    # 4 query chunks per batch
ROPE_BASE = 10000.0
SCALE = 1.0 / 8.0                # 1/sqrt(HEAD_DIM)

_PROGRAM = None


def _build_program():
    nc = bacc.Bacc(None, target_bir_lowering=False)

    xT = nc.dram_tensor("xT", [C, NT], F32R, kind="ExternalInput")
    wq = nc.dram_tensor("wq", [C, HF], F32R, kind="ExternalInput")
    wk = nc.dram_tensor("wk", [C, HF], F32R, kind="ExternalInput")
    wv = nc.dram_tensor("wv", [C, HF], F32R, kind="ExternalInput")
    wp = nc.dram_tensor("wp", [HF, C], F32R, kind="ExternalInput")
    identd = nc.dram_tensor("identd", [128, 128], F32R, kind="ExternalInput")
    onesdd = nc.dram_tensor("onesdd", [128, 64], F32R, kind="ExternalInput")
    cosd = nc.dram_tensor("cosd", [HF, T], F32R, kind="ExternalInput")
    ssind = nc.dram_tensor("ssind", [HF, T], F32R, kind="ExternalInput")
    outT = nc.dram_tensor("outT", [C, NT], F32, kind="ExternalOutput")

    with tile.TileContext(nc) as tc:
        with (
            tc.tile_pool(name="const", bufs=1) as cpool,
            tc.tile_pool(name="sx", bufs=2) as sx,
            tc.tile_pool(name="srope", bufs=1) as srope,
            tc.tile_pool(name="schunk", bufs=3) as schunk,
            tc.tile_pool(name="sv", bufs=2) as sv,
            tc.tile_pool(name="spt", bufs=4) as spt,
            tc.tile_pool(name="sy", bufs=2) as sy,
            tc.tile_pool(name="sst", bufs=2) as sst,
            tc.tile_pool(name="pmm", bufs=2, space="PSUM") as pmm,
            tc.tile_pool(name="psc", bufs=2, space="PSUM") as psc,
            tc.tile_pool(name="py", bufs=2, space="PSUM") as py,
        ):
            # ---- constants ----
            ident = cpool.tile([128, 128], F32R, tag="ident")
            nc.gpsimd.dma_start(ident[:], identd[:])
            onesd = cpool.tile([128, 64], F32R, tag="onesd")
            nc.gpsimd.dma_start(onesd[:], onesdd[:])

            wqs = cpool.tile([128, C], F32R, tag="wqs")
            wks = cpool.tile([128, C], F32R, tag="wks")
            wvs = cpool.tile([128, C], F32R, tag="wvs")
            for kt in range(KT):
                nc.gpsimd.dma_start(wqs[:, kt * HF:(kt + 1) * HF], wq[kt * 128:(kt + 1) * 128, :])
                nc.gpsimd.dma_start(wks[:, kt * HF:(kt + 1) * HF], wk[kt * 128:(kt + 1) * 128, :])
                nc.gpsimd.dma_start(wvs[:, kt * HF:(kt + 1) * HF], wv[kt * 128:(kt + 1) * 128, :])
            wps = cpool.tile([128, C], F32R, tag="wps")
            nc.gpsimd.dma_start(wps[:], wp[:])
            cost = cpool.tile([128, T], F32R, tag="cost")
            nc.gpsimd.dma_start(cost[:], cosd[:])
            ssint = cpool.tile([128, T], F32R, tag="ssint")
            nc.gpsimd.dma_start(ssint[:], ssind[:])

            for b in range(B):
                boff = b * T
                # ---- fused qkv projection + rope, per 512-token chunk ----
                rope_q = srope.tile([128, T], F32R, tag="rope_q")
                rope_k = srope.tile([128, T], F32R, tag="rope_k")
                vT = srope.tile([128, T], F32R, tag="vT")
                for c in range(NQC):
                    cc = slice(c * QC, (c + 1) * QC)
                    xs = sx.tile([128, KT * QC], F32R, tag="xs")
                    for kt in range(KT):
                        nc.sync.dma_start(
                            xs[:, kt * QC:(kt + 1) * QC],
                            xT[kt * 128:(kt + 1) * 128, boff + c * QC: boff + (c + 1) * QC],
                        )
                    for wslab, dest in ((wqs, None), (wks, None), (wvs, vT)):
                        ps = pmm.tile([128, QC], F32, tag="mm")
                        for kt in range(KT):
                            nc.tensor.matmul(
                                ps[:], wslab[:, kt * HF:(kt + 1) * HF],
                                xs[:, kt * QC:(kt + 1) * QC],
                                start=(kt == 0), stop=(kt == KT - 1),
                            )
                        if dest is not None:
                            nc.vector.tensor_copy(dest[:, cc], ps[:])
                            continue
                        dst = rope_q if wslab is wqs else rope_k
                        raw = schunk.tile([128, QC], F32R, tag="rawc")
                        ta = schunk.tile([128, QC], F32R, tag="tac")
                        nc.vector.tensor_copy(raw[:], ps[:])
                        nc.vector.tensor_mul(ta[:], ps[:], cost[:, cc])
                        shuf = schunk.tile([128, QC], F32R, tag="shufc")
                        for h in range(HPC):
                            hb = h * 64
                            nc.scalar.dma_start(shuf[hb:hb + 32, :], raw[hb + 32:hb + 64, :])
                            nc.scalar.dma_start(shuf[hb + 32:hb + 64, :], raw[hb:hb + 32, :])
                        tb = schunk.tile([128, QC], F32R, tag="tbc")
                        nc.vector.tensor_mul(tb[:], shuf[:], ssint[:, cc])
                        nc.gpsimd.tensor_add(dst[:, cc], ta[:], tb[:])

                # ---- v -> natural layout [tok, feat] + ones cols ----
                # per ki (stride 130): [h0 v (64) | ones | h1 v (64) | ones]
                vv = sv.tile([128, (T // 128) * 130], F32R, tag="vv")
                nc.sync.dma_start(vv[:, 64:(T // 128) * 130:65], onesd[:, 0:32])
                for ki in range(T // 128):
                    pst = pmm.tile([128, QC], F32R, tag="mm")
                    nc.tensor.transpose(pst[:, 0:128], vT[:, ki * 128:(ki + 1) * 128], ident[:])
                    nc.vector.tensor_copy(vv[:, ki * 130:ki * 130 + 64], pst[:, 0:64])
                    nc.vector.tensor_copy(vv[:, ki * 130 + 65:ki * 130 + 129], pst[:, 64:128])

                # ---- attention ----
                ynorm = sy.tile([128, T], F32R, tag="ynorm")
                for qc in range(NQC):
                    qs = qc * QC
                    nki = (qs + QC) // 128
                    y_aug = [py.tile([65, QC], F32, tag="y", name=f"yaug{h}") for h in range(HPC)]
                    for pr in range(nki // 2):
                        kia, kib = 2 * pr, 2 * pr + 1
                        c0a = max(0, kia * 128 - qs)
                        c0b = max(0, kib * 128 - qs)
                        for h in range(HPC):
                            hb = h * 64
                            sc = psc.tile([128, 2 * QC], F32, tag="sc")
                            nc.tensor.matmul(
                                sc[:, c0a:QC],
                                rope_k[hb:hb + 64, kia * 128:(kia + 1) * 128],
                                rope_q[hb:hb + 64, qs + c0a:qs + QC],
                                start=True, stop=True,
                            )
                            nc.tensor.matmul(
                                sc[:, QC + c0b:2 * QC],
                                rope_k[hb:hb + 64, kib * 128:(kib + 1) * 128],
                                rope_q[hb:hb + 64, qs + c0b:qs + QC],
                                start=True, stop=True,
                            )
                            pt = spt.tile([128, 2 * QC], F32R, tag="pt")
                            nc.scalar.activation(
                                pt[:, c0a:2 * QC], sc[:, c0a:2 * QC], AF.Exp, scale=SCALE,
                            )
                            for ki, half, c0 in ((kia, 0, c0a), (kib, 1, c0b)):
                                if ki * 128 >= qs:  # diagonal block: zero where k > q
                                    nc.gpsimd.affine_select(
                                        pt[:, half * QC + c0: half * QC + c0 + 128],
                                        pt[:, half * QC + c0: half * QC + c0 + 128],
                                        pattern=[[1, 128]],
                                        compare_op=mybir.AluOpType.is_ge,
                                        fill=0.0, base=0, channel_multiplier=-1,
                                    )
                                nc.tensor.matmul(
                                    y_aug[h][0:65, c0:QC],
                                    vv[:, ki * 130 + 65 * h: ki * 130 + 65 * h + 65],
                                    pt[:, half * QC + c0: (half + 1) * QC],
                                    start=(pr == 0 and half == 0),
                                    stop=(pr == nki // 2 - 1 and half == 1),
                                )
                    for h in range(HPC):
                        rec = sst.tile([128, QC], F32R, tag="rec")
                        with nc.allow_low_precision(reason="f32r is f32-width"):
                            nc.vector.reciprocal(rec[64:65, :], y_aug[h][64:65, :])
                        bc = pmm.tile([128, QC], F32, tag="mm")
                        nc.tensor.matmul(
                            bc[0:64, :], onesd[64:65, 0:64], rec[64:65, :],
                            start=True, stop=True,
                        )
                        bcs = sst.tile([64, QC], F32, tag="bcs")
                        nc.vector.tensor_copy(bcs[:], bc[0:64, :])
                        if h == 0:
                            nc.vector.tensor_mul(
                                ynorm[0:64, qs:qs + QC], y_aug[h][0:64, :], bcs[:])
                        else:
                            hn = sst.tile([64, QC], F32R, tag="hn")
                            nc.vector.tensor_mul(hn[:], y_aug[h][0:64, :], bcs[:])
                            nc.sync.dma_start(ynorm[64:128, qs:qs + QC], hn[:])

                # ---- output projection (out^T partial) ----
                for of in range(KT):
                    for c in range(NQC):
                        op = pmm.tile([128, QC], F32, tag="mm")
                        nc.tensor.matmul(
                            op[:], wps[:, of * 128:(of + 1) * 128],
                            ynorm[:, c * QC:(c + 1) * QC],
                            start=True, stop=True,
                        )
                        st = sst.tile([128, QC], F32, tag="st")
                        if (of + c) % 2 == 0:
                            nc.vector.tensor_copy(st[:], op[:])
                        else:
                            nc.scalar.copy(st[:], op[:])
                        nc.sync.dma_start(
                            outT[of * 128:(of + 1) * 128, boff + c * QC: boff + (c + 1) * QC],
                            st[:],
                        )
    nc.finalize()
    return nc


def _program():
    global _PROGRAM
    if _PROGRAM is None:
        _PROGRAM = _build_program()
    return _PROGRAM


def _rope_tables():
    inv_freq = 1.0 / (ROPE_BASE ** (np.arange(0, HEAD_DIM, 2, dtype=np.float32) / HEAD_DIM))
    t = np.arange(T, dtype=np.float32)
    freqs = np.outer(t, inv_freq).astype(np.float32)        # [T, 32]
    emb = np.concatenate([freqs, freqs], axis=1)            # [T, 64]
    cos = np.cos(emb).astype(np.float32)                    # [T, 64]
    sin = np.sin(emb).astype(np.float32)
    ssin = sin.copy()
    ssin[:, :32] *= -1.0                                    # signed for rotate_half
    cosT = np.ascontiguousarray(cos.T)                      # [64, T]
    ssinT = np.ascontiguousarray(ssin.T)
    cos2 = np.concatenate([cosT] * HPC, axis=0)             # [128, T]
    ssin2 = np.concatenate([ssinT] * HPC, axis=0)
    return cos2, ssin2


def _prep_in_maps(x, W_attn, W_proj):
    x = np.asarray(x, dtype=np.float32)
    W_attn = np.asarray(W_attn, dtype=np.float32)
    W_proj = np.asarray(W_proj, dtype=np.float32)
    xT = np.ascontiguousarray(x.reshape(NT, C).T)
    cos2, ssin2 = _rope_tables()
    in_maps = []
    for i in range(N_CORES):
        cs = i * HF
        in_maps.append({
            "xT": xT,
            "wq": np.ascontiguousarray(W_attn[:, cs:cs + HF]),
            "wk": np.ascontiguousarray(W_attn[:, C + cs:C + cs + HF]),
            "wv": np.ascontiguousarray(W_attn[:, 2 * C + cs:2 * C + cs + HF]),
            "wp": np.ascontiguousarray(W_proj[cs:cs + HF, :]),
            "identd": np.eye(128, dtype=np.float32),
            "onesdd": np.ones((128, 64), dtype=np.float32),
            "cosd": cos2,
            "ssind": ssin2,
        })
    return in_maps


def _run(in_maps, trace=False, **kwargs):
    return run_bass_kernel_spmd(
        _program(), in_maps, core_ids=list(range(N_CORES)), trace=trace, **kwargs
    )


def kernel(x, W_attn, W_proj):
    in_maps = _prep_in_maps(x, W_attn, W_proj)
    res = _run(in_maps)
    acc = np.zeros((C, NT), dtype=np.float32)
    for r in res.results:
        acc += r["outT"]
    return np.ascontiguousarray(acc.T).reshape(B, T, C)
